# revision 17
# baseline (speedup 1.0000x reference)
"""Trainium2 Bass kernel for pointer-generator attention with coverage.

reference math (per batch row b):
    enc = h[b] @ W_h.T                      # [T, N]
    dec = s_t_hat[b] @ W_dec.T + b_dec      # [N]
    e   = tanh(enc + dec + coverage[b,:,None]*w_c)   # [T, N]
    s   = e @ v                             # [T]
    w   = exp(s) * mask[b]                  # (softmax w/o max-sub; scores are O(1))
    attn= w / sum(w)
    c_t = attn @ h[b]                       # [N]
    cov_new = coverage[b] + attn

Distribution: pure data parallel, 32 batches / 8 cores = 4 per core. No
collectives. Inside a core, everything is laid out feature-major for the
big matmul: e is computed as e^T tiles [n-chunk(128 part), t-span(512)].

The PE does only matmuls, in bf16: h and the weights are cast to bf16
DRAM scratch by SWDGE cast-DMAs, and the contraction-axis transposes
(h's and W's contiguous axis) are done by the DMA xbar
(dma_start_transpose) on the way into SBUF. The big matmul processes
two 512-spans per stationary weight tile (halves LDWEIGHTS), the cov
term folds in as a K=1 contraction row, dec folds into the tanh as the
ScalarE per-partition bias, and the scores matmul is emitted one
m-group late so the in-order PE queue never stalls on the tanh
(periodic sub-us stalls keep the HAM clock gate at 1.2GHz otherwise).
c_t uses the attn column as a 1-column stationary against natural bf16
h tiles, accumulating [1, N] rows directly.
"""

import os
import sys

sys.path.insert(0, "/opt/trn_rl_repo")

import numpy as np

import concourse.bass as bass
import concourse.bacc as bacc
import concourse.mybir as mybir
from concourse import masks
from concourse import tile

F32 = mybir.dt.float32
BF16 = mybir.dt.bfloat16
AF = mybir.ActivationFunctionType
ALU = mybir.AluOpType
AX = mybir.AxisListType

N_CORES = 8


def _enable_ldw_opt():
    """Flip walrus's LDWEIGHTS scheduling optimization on (the default
    compile flags carry --enable-ldw-opt=false)."""
    try:
        from concourse import compiler_utils

        flags = compiler_utils.get_compiler_flags()
        flags = [
            f.replace("--enable-ldw-opt=false", "--enable-ldw-opt=true")
            for f in flags
        ]
        compiler_utils.set_compiler_flags(flags)
    except Exception:
        pass


def build_graph(B=4, T=2048, N=1024):
    """One core's graph: B batch rows of a [B,T,N] problem."""
    NC_ = N // 128          # feature chunks
    TJ = T // 128           # t-chunks of 128
    TS = T // 512           # t-spans of 512
    assert N % 128 == 0 and T % 512 == 0
    pairs = [list(range(p, min(p + 2, TS))) for p in range(0, TS, 2)]
    halves = [(i * 512, min(512, N - i * 512)) for i in range((N + 511) // 512)]
    assert len(halves) <= 2

    nc = bacc.Bacc(None, target_bir_lowering=False)

    s_t_hat = nc.declare_dram_parameter("s_t_hat", [B, N], F32, False)
    h = nc.declare_dram_parameter("h", [B, T, N], F32, False)
    mask_d = nc.declare_dram_parameter("mask", [B, T], F32, False)
    cov_d = nc.declare_dram_parameter("coverage", [B, T], F32, False)
    W_h = nc.declare_dram_parameter("W_h", [N, N], F32, False)
    W_dec = nc.declare_dram_parameter("W_dec", [N, N], F32, False)
    b_dec = nc.declare_dram_parameter("b_dec", [1, N], F32, False)
    w_c = nc.declare_dram_parameter("w_c", [1, N], F32, False)
    v_d = nc.declare_dram_parameter("v", [1, N], F32, False)
    ct_out = nc.declare_dram_parameter("ct", [B, N], F32, isOutput=True)
    attn_out = nc.declare_dram_parameter("attn", [B, T], F32, isOutput=True)
    covnew_out = nc.declare_dram_parameter("covnew", [B, T], F32, isOutput=True)

    with tile.TileContext(nc) as tc:
        with (
            tc.tile_pool(name="const", bufs=1) as constp,
            tc.tile_pool(name="wht", bufs=1) as whtp,
            tc.tile_pool(name="small", bufs=1) as smallp,
            tc.tile_pool(name="dram", bufs=1, space="DRAM") as dramp,
            tc.tile_pool(name="ps_e", bufs=2, space="PSUM") as ps_e,
            tc.tile_pool(name="ps_s", bufs=2, space="PSUM") as ps_s,
            tc.tile_pool(name="ps_m", bufs=2, space="PSUM") as ps_m,
        ):
            # ---- bf16 DRAM scratch; start the big h casts first ----
            wbf = dramp.tile([N, N], BF16, tag="wbf")
            nc.gpsimd.dma_start(out=wbf[:], in_=W_h[:, :])
            wdbf = dramp.tile([N, N], BF16, tag="wdbf")
            nc.gpsimd.dma_start(out=wdbf[:], in_=W_dec[:, :])
            hbfs = [
                dramp.tile([T, N], BF16, tag=f"hbf{b}", name=f"hbf{b}")
                for b in range(B)
            ]
            for b in range(B):
                for s in range(TS):
                    nc.gpsimd.dma_start(
                        out=hbfs[b][s * 512 : (s + 1) * 512, :],
                        in_=h[b, s * 512 : (s + 1) * 512, :],
                    )

            ident = constp.tile([128, 128], F32, tag="ident")
            masks.make_identity(nc, ident[:])

            # ---- persistent small tensors ----
            wc_sb = smallp.tile([1, N], F32, tag="wc")
            nc.sync.dma_start(out=wc_sb[:], in_=w_c[:, :])
            wc_r = smallp.tile([1, N], BF16, tag="wcr")
            nc.vector.tensor_copy(wc_r[:], wc_sb[:])
            bdec_sb = smallp.tile([1, N], F32, tag="bdec")
            nc.sync.dma_start(out=bdec_sb[:], in_=b_dec[:, :])
            bdec_b = smallp.tile([1, N], BF16, tag="bdecb")
            nc.vector.tensor_copy(bdec_b[:], bdec_sb[:])
            v_sb = smallp.tile([1, N], F32, tag="v")
            nc.sync.dma_start(out=v_sb[:], in_=v_d[:, :])
            snat = smallp.tile([B, N], F32, tag="snat")
            nc.sync.dma_start(out=snat[:], in_=s_t_hat[:, :])
            ones1B = smallp.tile([1, B], BF16, tag="ones1B")
            nc.vector.memset(ones1B[:], 1.0)

            # W_h^T via cast + xbar transpose: whts[p, k, n] = W_h[n, k*128+p]
            whts = whtp.tile([128, NC_, N], BF16, tag="whts")
            nc.sync.dma_start_transpose(whts[:], wbf[:])

            vT = smallp.tile([128, NC_], BF16, tag="vT")
            stT = smallp.tile([128, B * NC_], BF16, tag="stT")
            dec_sb = smallp.tile([B, N], F32, tag="dec")
            dec_fm = smallp.tile([128, NC_ * B], F32, tag="decfm")

            # ---- setup: s_t_hat^T, v^T, dec ----
            for k in range(NC_):
                tp = ps_m.tile([128, 16], F32, tag="mp")
                nc.tensor.transpose(
                    tp[:, 0:B], snat[:, k * 128 : (k + 1) * 128], ident[:B, :B]
                )
                nc.vector.tensor_copy(stT[:, k * B : (k + 1) * B], tp[:, 0:B])
            for m in range(NC_):
                tp = ps_m.tile([128, 16], F32, tag="mp")
                nc.tensor.transpose(
                    tp[:, 0:1], v_sb[:, m * 128 : (m + 1) * 128], ident[:1, :1]
                )
                nc.vector.tensor_copy(vT[:, m : m + 1], tp[:, 0:1])

            with tc.tile_pool(name="setup", bufs=1) as setupp:
                wdts = setupp.tile([128, NC_, N], BF16, tag="wdts")
                nc.sync.dma_start_transpose(wdts[:], wdbf[:])
                dpss = [
                    ps_s.tile([B, 512], F32, tag="sp", name=f"dps{i}")
                    for i in range(len(halves))
                ]
                for k in range(NC_):
                    for i, (h0, hw) in enumerate(halves):
                        nc.tensor.matmul(
                            dpss[i][:, 0:hw],
                            stT[:, k * B : (k + 1) * B],
                            wdts[:, k, h0 : h0 + hw],
                            start=(k == 0),
                            stop=False,
                        )
                for i, (h0, hw) in enumerate(halves):
                    nc.tensor.matmul(
                        dpss[i][:, 0:hw],
                        ones1B[:],
                        bdec_b[:, h0 : h0 + hw],
                        start=False,
                        stop=True,
                    )
                    nc.scalar.copy(dec_sb[:, h0 : h0 + hw], dpss[i][:, 0:hw])
                for m in range(NC_):
                    tp = ps_m.tile([128, 16], F32, tag="mp")
                    nc.tensor.transpose(
                        tp[:, 0:B], dec_sb[:, m * 128 : (m + 1) * 128], ident[:B, :B]
                    )
                    nc.vector.tensor_copy(dec_fm[:, m * B : (m + 1) * B], tp[:, 0:B])

            # ================= main per-batch pipeline =================
            with (
                tc.tile_pool(name="hts", bufs=2) as htsp,
                tc.tile_pool(name="hbn", bufs=3) as hbnp,
                tc.tile_pool(name="esb", bufs=6) as esbp,
                tc.tile_pool(name="rows", bufs=2) as rowp,
            ):

                def pair_TM(b, pair, covrow_b, scores_sb, hT, deferred):
                    """matmul + tanh + scores for a pair of 512-spans,
                    reusing each stationary weight tile across the pair."""
                    scpss = [
                        ps_s.tile([B, 512], F32, tag="sp", name=f"scps{i}")
                        for i in range(len(pair))
                    ]
                    for m in range(NC_):
                        epss = [
                            ps_e.tile([128, 512], F32, tag=f"ep{i}", name=f"eps{i}")
                            for i in range(len(pair))
                        ]
                        for k in range(NC_):
                            for i, s in enumerate(pair):
                                nc.tensor.matmul(
                                    epss[i][:],
                                    whts[:, k, m * 128 : (m + 1) * 128],
                                    hT[:, k, s * 512 : (s + 1) * 512],
                                    start=(k == 0),
                                    stop=False,
                                )
                        # cov fold: + w_c[n] * cov[t]
                        for i, s in enumerate(pair):
                            nc.tensor.matmul(
                                epss[i][:],
                                wc_r[:, m * 128 : (m + 1) * 128],
                                covrow_b[:, s * 512 : (s + 1) * 512],
                                start=False,
                                stop=True,
                            )
                        ets = []
                        for i, s in enumerate(pair):
                            et = esbp.tile([128, 512], BF16, tag="et")
                            nc.scalar.activation(
                                et[:],
                                epss[i][:],
                                AF.Tanh,
                                bias=dec_fm[:, m * B + b : m * B + b + 1],
                            )
                            ets.append(et)

                        # scores matmul emitted one m-group late so the
                        # in-order PE queue never waits on the ACT tanh
                        def mk(mm_m=m, mm_ets=ets, mm_scpss=scpss, mm_pair=pair):
                            def emit():
                                for i, s in enumerate(mm_pair):
                                    nc.tensor.matmul(
                                        mm_scpss[i][0:1, :],
                                        vT[:, mm_m : mm_m + 1],
                                        mm_ets[i][:],
                                        start=(mm_m == 0),
                                        stop=(mm_m == NC_ - 1),
                                    )
                                if mm_m == NC_ - 1:
                                    for i, s in enumerate(mm_pair):
                                        nc.vector.tensor_copy(
                                            scores_sb[:, s * 512 : (s + 1) * 512],
                                            mm_scpss[i][0:1, :],
                                        )

                            return emit

                        deferred.append(mk())
                        if len(deferred) > 1:
                            deferred.pop(0)()

                def phase_SC(b, covrow, scores_sb, hbf):
                    """softmax (in place on scores_sb), outputs, c_t."""
                    maskrow = rowp.tile([1, T], F32, tag="maskrow", bufs=1)
                    nc.sync.dma_start(out=maskrow[:], in_=mask_d[b : b + 1, :])
                    nc.scalar.activation(scores_sb[:], scores_sb[:], AF.Exp)
                    nc.vector.tensor_mul(scores_sb[:], scores_sb[:], maskrow[:])
                    ssum = smallp.tile([1, 1], F32, tag="ssum")
                    nc.vector.tensor_reduce(ssum[:], scores_sb[:], AX.X, ALU.add)
                    sinv = smallp.tile([1, 1], F32, tag="sinv")
                    nc.vector.reciprocal(sinv[:], ssum[:])
                    nc.vector.tensor_scalar_mul(scores_sb[:], scores_sb[:], sinv[:])
                    attn_sb = scores_sb  # now holds attn
                    nc.sync.dma_start(out=attn_out[b : b + 1, :], in_=attn_sb[:])
                    cn = rowp.tile([1, T], F32, tag="cn", bufs=1)
                    nc.vector.tensor_add(cn[:], covrow[:], attn_sb[:])
                    nc.sync.dma_start(out=covnew_out[b : b + 1, :], in_=cn[:])
                    atp = ps_m.tile([128, 16], F32, tag="mp")
                    for j in range(TJ):
                        nc.tensor.transpose(
                            atp[:, j : j + 1],
                            attn_sb[:, j * 128 : (j + 1) * 128],
                            ident[:1, :1],
                        )
                    attnT = smallp.tile([128, TJ], BF16, tag="attnT", bufs=2)
                    nc.vector.tensor_copy(attnT[:], atp[:, 0:TJ])

                    # c_t = attn @ h: attn column is the (1-col) stationary,
                    # natural bf16 h tiles are the 512-wide moving operand.
                    ctps = [
                        ps_m.tile([1, 512], F32, tag="mp", name=f"ctps{i}")
                        for i in range(len(halves))
                    ]
                    for jg in range(TS):
                        hb = hbnp.tile([128, 4 * N], BF16, tag="hb")
                        nc.sync.dma_start(
                            out=hb[:].rearrange("p (j k) -> p j k", j=4),
                            in_=hbf[jg * 512 : (jg + 1) * 512, :].rearrange(
                                "(j p) k -> p j k", p=128
                            ),
                        )
                        for j in range(4):
                            jj = jg * 4 + j
                            for i, (h0, hw) in enumerate(halves):
                                nc.tensor.matmul(
                                    ctps[i][:, 0:hw],
                                    attnT[:, jj : jj + 1],
                                    hb[:, j * N + h0 : j * N + h0 + hw],
                                    start=(jj == 0),
                                    stop=(jj == TJ - 1),
                                )
                    ctrow = smallp.tile([1, N], F32, tag="ctrow", bufs=2)
                    for i, (h0, hw) in enumerate(halves):
                        nc.vector.tensor_copy(ctrow[:, h0 : h0 + hw], ctps[i][:, 0:hw])
                    nc.sync.dma_start(out=ct_out[b : b + 1, :], in_=ctrow[:])

                # software pipeline: batch b's softmax/c_t is emitted after
                # batch b+1's first span-pair so PE has matmul work queued
                # while the softmax chain runs on ACT/DVE.
                pend = None
                deferred = []
                for b in range(B):
                    hT = htsp.tile([128, NC_, T], BF16, tag="hT")
                    for s in range(TS):
                        nc.sync.dma_start_transpose(
                            hT[:, :, s * 512 : (s + 1) * 512],
                            hbfs[b][s * 512 : (s + 1) * 512, :],
                        )
                    covrow = rowp.tile([1, T], F32, tag="covrow")
                    nc.sync.dma_start(out=covrow[:], in_=cov_d[b : b + 1, :])
                    covrow_b = rowp.tile([1, T], BF16, tag="covrowb")
                    nc.vector.tensor_copy(covrow_b[:], covrow[:])
                    scores_sb = rowp.tile([1, T], F32, tag="scores")
                    for pi, pair in enumerate(pairs):
                        pair_TM(b, pair, covrow_b, scores_sb, hT, deferred)
                        if pi == 0 and pend is not None:
                            phase_SC(*pend)
                            pend = None
                    while deferred:
                        deferred.pop(0)()
                    pend = (b, covrow, scores_sb, hbfs[b])
                phase_SC(*pend)

    return nc


_CACHE = {}


def _get_graph():
    if "nc" not in _CACHE:
        _enable_ldw_opt()
        nc = build_graph()
        nc.finalize()
        _CACHE["nc"] = nc
    return _CACHE["nc"]


def kernel(s_t_hat, h, enc_padding_mask, coverage, W_h, W_dec, b_dec, w_c, v, **_):
    from concourse.bass_utils import run_bass_kernel_spmd

    f = lambda x: np.ascontiguousarray(np.asarray(x), dtype=np.float32)
    s_t_hat, h = f(s_t_hat), f(h)
    enc_padding_mask, coverage = f(enc_padding_mask), f(coverage)
    W_h, W_dec = f(W_h), f(W_dec)
    b_dec, w_c, v = (
        f(b_dec).reshape(1, -1),
        f(w_c).reshape(1, -1),
        f(v).reshape(1, -1),
    )

    Btot = h.shape[0]
    Bper = Btot // N_CORES
    nc = _get_graph()
    in_maps = []
    for i in range(N_CORES):
        sl = slice(i * Bper, (i + 1) * Bper)
        in_maps.append(
            {
                "s_t_hat": s_t_hat[sl],
                "h": h[sl],
                "mask": enc_padding_mask[sl],
                "coverage": coverage[sl],
                "W_h": W_h,
                "W_dec": W_dec,
                "b_dec": b_dec,
                "w_c": w_c,
                "v": v,
            }
        )
    res = run_bass_kernel_spmd(
        nc,
        in_maps,
        core_ids=list(range(N_CORES)),
        trace=bool(os.environ.get("KERNEL_TRACE")),
    )
    kernel.last_results = res
    rs = res.results
    ct = np.concatenate([r["ct"] for r in rs], 0)
    attn = np.concatenate([r["attn"] for r in rs], 0)
    covnew = np.concatenate([r["covnew"] for r in rs], 0)
    return ct, attn, covnew


# revision 18
# speedup vs baseline: 1.2739x; 1.2739x over previous
"""Trainium2 Bass kernel for pointer-generator attention with coverage.

reference math (per batch row b):
    enc = h[b] @ W_h.T                      # [T, N]
    dec = s_t_hat[b] @ W_dec.T + b_dec      # [N]
    e   = tanh(enc + dec + coverage[b,:,None]*w_c)   # [T, N]
    s   = e @ v                             # [T]
    w   = exp(s) * mask[b]                  # (softmax w/o max-sub; scores are O(1))
    attn= w / sum(w)
    c_t = attn @ h[b]                       # [N]
    cov_new = coverage[b] + attn

Distribution: pure data parallel, 32 batches / 8 cores = 4 per core. No
collectives. Inside a core, everything is laid out feature-major for the
big matmul: e is computed as e^T tiles [n-chunk(128 part), t-span(512)].

The PE does only matmuls, in bf16: h and the weights are cast to bf16
DRAM scratch by SWDGE cast-DMAs, and the contraction-axis transposes
(h's and W's contiguous axis) are done by the DMA xbar
(dma_start_transpose) on the way into SBUF. The big matmul processes
two 512-spans per stationary weight tile (halves LDWEIGHTS), the cov
term folds in as a K=1 contraction row, dec folds into the tanh as the
ScalarE per-partition bias, and the scores matmul is emitted one
m-group late so the in-order PE queue never stalls on the tanh
(periodic sub-us stalls keep the HAM clock gate at 1.2GHz otherwise).
c_t uses the attn column as a 1-column stationary against natural bf16
h tiles, accumulating [1, N] rows directly.
"""

import os
import sys

sys.path.insert(0, "/opt/trn_rl_repo")

import numpy as np

import concourse.bass as bass
import concourse.bacc as bacc
import concourse.mybir as mybir
from concourse import masks
from concourse import tile

F32 = mybir.dt.float32
BF16 = mybir.dt.bfloat16
AF = mybir.ActivationFunctionType
ALU = mybir.AluOpType
AX = mybir.AxisListType

N_CORES = 8


def _enable_ldw_opt():
    """Flip walrus's LDWEIGHTS scheduling optimization on (the default
    compile flags carry --enable-ldw-opt=false)."""
    try:
        from concourse import compiler_utils

        flags = compiler_utils.get_compiler_flags()
        flags = [
            f.replace("--enable-ldw-opt=false", "--enable-ldw-opt=true")
            for f in flags
        ]
        compiler_utils.set_compiler_flags(flags)
    except Exception:
        pass


def build_graph(B=4, T=2048, N=1024):
    """One core's graph: B batch rows of a [B,T,N] problem."""
    NC_ = N // 128          # feature chunks
    TJ = T // 128           # t-chunks of 128
    TS = T // 512           # t-spans of 512
    assert N % 128 == 0 and T % 512 == 0
    pairs = [list(range(p, min(p + 2, TS))) for p in range(0, TS, 2)]
    halves = [(i * 512, min(512, N - i * 512)) for i in range((N + 511) // 512)]
    assert len(halves) <= 2

    nc = bacc.Bacc(None, target_bir_lowering=False)

    s_t_hat = nc.declare_dram_parameter("s_t_hat", [B, N], F32, False)
    h = nc.declare_dram_parameter("h", [B, T, N], F32, False)
    mask_d = nc.declare_dram_parameter("mask", [B, T], F32, False)
    cov_d = nc.declare_dram_parameter("coverage", [B, T], F32, False)
    W_h = nc.declare_dram_parameter("W_h", [N, N], F32, False)
    W_dec = nc.declare_dram_parameter("W_dec", [N, N], F32, False)
    b_dec = nc.declare_dram_parameter("b_dec", [1, N], F32, False)
    w_c = nc.declare_dram_parameter("w_c", [1, N], F32, False)
    v_d = nc.declare_dram_parameter("v", [1, N], F32, False)
    ct_out = nc.declare_dram_parameter("ct", [B, N], F32, isOutput=True)
    attn_out = nc.declare_dram_parameter("attn", [B, T], F32, isOutput=True)
    covnew_out = nc.declare_dram_parameter("covnew", [B, T], F32, isOutput=True)

    with tile.TileContext(nc) as tc:
        with (
            tc.tile_pool(name="const", bufs=1) as constp,
            tc.tile_pool(name="wht", bufs=1) as whtp,
            tc.tile_pool(name="small", bufs=1) as smallp,
            tc.tile_pool(name="dram", bufs=1, space="DRAM") as dramp,
            tc.tile_pool(name="ps_e", bufs=2, space="PSUM") as ps_e,
            tc.tile_pool(name="ps_s", bufs=2, space="PSUM") as ps_s,
            tc.tile_pool(name="ps_m", bufs=2, space="PSUM") as ps_m,
        ):
            # ---- bf16 DRAM scratch; start the big h casts first ----
            wbf = dramp.tile([N, N], BF16, tag="wbf")
            nc.gpsimd.dma_start(out=wbf[:], in_=W_h[:, :])
            wdbf = dramp.tile([N, N], BF16, tag="wdbf")
            nc.gpsimd.dma_start(out=wdbf[:], in_=W_dec[:, :])
            hbfs = [
                dramp.tile([T, N], BF16, tag=f"hbf{b}", name=f"hbf{b}")
                for b in range(B)
            ]

            ident = constp.tile([128, 128], F32, tag="ident")
            masks.make_identity(nc, ident[:])

            # ---- persistent small tensors ----
            wc_sb = smallp.tile([1, N], F32, tag="wc")
            nc.sync.dma_start(out=wc_sb[:], in_=w_c[:, :])
            wc_r = smallp.tile([1, N], BF16, tag="wcr")
            nc.vector.tensor_copy(wc_r[:], wc_sb[:])
            bdec_sb = smallp.tile([1, N], F32, tag="bdec")
            nc.sync.dma_start(out=bdec_sb[:], in_=b_dec[:, :])
            bdec_b = smallp.tile([1, N], BF16, tag="bdecb")
            nc.vector.tensor_copy(bdec_b[:], bdec_sb[:])
            v_sb = smallp.tile([1, N], F32, tag="v")
            nc.sync.dma_start(out=v_sb[:], in_=v_d[:, :])
            snat = smallp.tile([B, N], F32, tag="snat")
            nc.sync.dma_start(out=snat[:], in_=s_t_hat[:, :])
            ones1B = smallp.tile([1, B], BF16, tag="ones1B")
            nc.vector.memset(ones1B[:], 1.0)

            # W_h^T via cast + xbar transpose: whts[p, k, n] = W_h[n, k*128+p]
            whts = whtp.tile([128, NC_, N], BF16, tag="whts")
            nc.sync.dma_start_transpose(whts[:], wbf[:])

            vT = smallp.tile([128, NC_], BF16, tag="vT")
            stT = smallp.tile([128, B * NC_], BF16, tag="stT")
            dec_sb = smallp.tile([B, N], F32, tag="dec")
            dec_fm = smallp.tile([128, NC_ * B], F32, tag="decfm")

            # ---- setup: s_t_hat^T, v^T, dec ----
            for k in range(NC_):
                tp = ps_m.tile([128, 16], F32, tag="mp")
                nc.tensor.transpose(
                    tp[:, 0:B], snat[:, k * 128 : (k + 1) * 128], ident[:B, :B]
                )
                nc.vector.tensor_copy(stT[:, k * B : (k + 1) * B], tp[:, 0:B])
            for m in range(NC_):
                tp = ps_m.tile([128, 16], F32, tag="mp")
                nc.tensor.transpose(
                    tp[:, 0:1], v_sb[:, m * 128 : (m + 1) * 128], ident[:1, :1]
                )
                nc.vector.tensor_copy(vT[:, m : m + 1], tp[:, 0:1])

            with tc.tile_pool(name="setup", bufs=1) as setupp:
                wdts = setupp.tile([128, NC_, N], BF16, tag="wdts")
                nc.sync.dma_start_transpose(wdts[:], wdbf[:])
                dpss = [
                    ps_s.tile([B, 512], F32, tag="sp", name=f"dps{i}")
                    for i in range(len(halves))
                ]
                for k in range(NC_):
                    for i, (h0, hw) in enumerate(halves):
                        nc.tensor.matmul(
                            dpss[i][:, 0:hw],
                            stT[:, k * B : (k + 1) * B],
                            wdts[:, k, h0 : h0 + hw],
                            start=(k == 0),
                            stop=False,
                        )
                for i, (h0, hw) in enumerate(halves):
                    nc.tensor.matmul(
                        dpss[i][:, 0:hw],
                        ones1B[:],
                        bdec_b[:, h0 : h0 + hw],
                        start=False,
                        stop=True,
                    )
                    nc.scalar.copy(dec_sb[:, h0 : h0 + hw], dpss[i][:, 0:hw])
                for m in range(NC_):
                    tp = ps_m.tile([128, 16], F32, tag="mp")
                    nc.tensor.transpose(
                        tp[:, 0:B], dec_sb[:, m * 128 : (m + 1) * 128], ident[:B, :B]
                    )
                    nc.vector.tensor_copy(dec_fm[:, m * B : (m + 1) * B], tp[:, 0:B])

            # ================= main per-batch pipeline =================
            with (
                tc.tile_pool(name="hts", bufs=2) as htsp,
                tc.tile_pool(name="hf32", bufs=2) as hf32p,
                tc.tile_pool(name="h16", bufs=2) as h16p,
                tc.tile_pool(name="hbn", bufs=2) as hbnp,
                tc.tile_pool(name="esb", bufs=6) as esbp,
                tc.tile_pool(name="rows", bufs=2) as rowp,
            ):

                def stage_pair(b, pair, hbf):
                    """h fp32 -> SBUF (HWDGE), cast to bf16 on DVE, store to
                    the DRAM scratch, then one xbar transpose-load for the
                    pair. (The SWDGE cast-DMA path runs at ~136GB/s and
                    rate-limits everything if h goes through it.)"""
                    t0 = pair[0] * 512
                    t1 = (pair[-1] + 1) * 512
                    for c0 in range(t0, t1, 256):
                        hf = hf32p.tile([128, 2 * N], F32, tag="hf")
                        nc.sync.dma_start(
                            out=hf[:].rearrange("p (j k) -> p j k", j=2),
                            in_=h[b, c0 : c0 + 256, :].rearrange(
                                "(j p) k -> p j k", p=128
                            ),
                        )
                        h16 = h16p.tile([128, 2 * N], BF16, tag="h16")
                        nc.vector.tensor_copy(h16[:], hf[:])
                        nc.sync.dma_start(
                            out=hbf[c0 : c0 + 256, :].rearrange(
                                "(j p) k -> p j k", p=128
                            ),
                            in_=h16[:].rearrange("p (j k) -> p j k", j=2),
                        )
                    hT = htsp.tile([128, NC_, t1 - t0], BF16, tag="hT")
                    nc.sync.dma_start_transpose(hT[:], hbf[t0:t1, :])
                    return hT

                def pair_TM(b, pair, covrow_b, scores_sb, hT, deferred):
                    """matmul + tanh + scores for a pair of 512-spans,
                    reusing each stationary weight tile across the pair."""
                    scpss = [
                        ps_s.tile([B, 512], F32, tag="sp", name=f"scps{i}")
                        for i in range(len(pair))
                    ]
                    for m in range(NC_):
                        epss = [
                            ps_e.tile([128, 512], F32, tag=f"ep{i}", name=f"eps{i}")
                            for i in range(len(pair))
                        ]
                        for k in range(NC_):
                            for i, s in enumerate(pair):
                                so = (s - pair[0]) * 512
                                nc.tensor.matmul(
                                    epss[i][:],
                                    whts[:, k, m * 128 : (m + 1) * 128],
                                    hT[:, k, so : so + 512],
                                    start=(k == 0),
                                    stop=False,
                                )
                        # cov fold: + w_c[n] * cov[t]
                        for i, s in enumerate(pair):
                            nc.tensor.matmul(
                                epss[i][:],
                                wc_r[:, m * 128 : (m + 1) * 128],
                                covrow_b[:, s * 512 : (s + 1) * 512],
                                start=False,
                                stop=True,
                            )
                        ets = []
                        for i, s in enumerate(pair):
                            et = esbp.tile([128, 512], BF16, tag="et")
                            nc.scalar.activation(
                                et[:],
                                epss[i][:],
                                AF.Tanh,
                                bias=dec_fm[:, m * B + b : m * B + b + 1],
                            )
                            ets.append(et)

                        # scores matmul emitted one m-group late so the
                        # in-order PE queue never waits on the ACT tanh
                        def mk(mm_m=m, mm_ets=ets, mm_scpss=scpss, mm_pair=pair):
                            def emit():
                                for i, s in enumerate(mm_pair):
                                    nc.tensor.matmul(
                                        mm_scpss[i][0:1, :],
                                        vT[:, mm_m : mm_m + 1],
                                        mm_ets[i][:],
                                        start=(mm_m == 0),
                                        stop=(mm_m == NC_ - 1),
                                    )
                                if mm_m == NC_ - 1:
                                    for i, s in enumerate(mm_pair):
                                        nc.vector.tensor_copy(
                                            scores_sb[:, s * 512 : (s + 1) * 512],
                                            mm_scpss[i][0:1, :],
                                        )

                            return emit

                        deferred.append(mk())
                        if len(deferred) > 1:
                            deferred.pop(0)()

                def phase_SC(b, covrow, scores_sb, hbf):
                    """softmax (in place on scores_sb), outputs, c_t."""
                    maskrow = rowp.tile([1, T], F32, tag="maskrow", bufs=1)
                    nc.sync.dma_start(out=maskrow[:], in_=mask_d[b : b + 1, :])
                    nc.scalar.activation(scores_sb[:], scores_sb[:], AF.Exp)
                    nc.vector.tensor_mul(scores_sb[:], scores_sb[:], maskrow[:])
                    ssum = smallp.tile([1, 1], F32, tag="ssum")
                    nc.vector.tensor_reduce(ssum[:], scores_sb[:], AX.X, ALU.add)
                    sinv = smallp.tile([1, 1], F32, tag="sinv")
                    nc.vector.reciprocal(sinv[:], ssum[:])
                    nc.vector.tensor_scalar_mul(scores_sb[:], scores_sb[:], sinv[:])
                    attn_sb = scores_sb  # now holds attn
                    nc.sync.dma_start(out=attn_out[b : b + 1, :], in_=attn_sb[:])
                    cn = rowp.tile([1, T], F32, tag="cn", bufs=1)
                    nc.vector.tensor_add(cn[:], covrow[:], attn_sb[:])
                    nc.sync.dma_start(out=covnew_out[b : b + 1, :], in_=cn[:])
                    atp = ps_m.tile([128, 16], F32, tag="mp")
                    for j in range(TJ):
                        nc.tensor.transpose(
                            atp[:, j : j + 1],
                            attn_sb[:, j * 128 : (j + 1) * 128],
                            ident[:1, :1],
                        )
                    attnT = smallp.tile([128, TJ], BF16, tag="attnT", bufs=2)
                    nc.vector.tensor_copy(attnT[:], atp[:, 0:TJ])

                    # c_t = attn @ h: attn column is the (1-col) stationary,
                    # natural bf16 h tiles are the 512-wide moving operand.
                    ctps = [
                        ps_m.tile([1, 512], F32, tag="mp", name=f"ctps{i}")
                        for i in range(len(halves))
                    ]
                    for jg in range(TS):
                        hb = hbnp.tile([128, 4 * N], BF16, tag="hb")
                        nc.sync.dma_start(
                            out=hb[:].rearrange("p (j k) -> p j k", j=4),
                            in_=hbf[jg * 512 : (jg + 1) * 512, :].rearrange(
                                "(j p) k -> p j k", p=128
                            ),
                        )
                        for j in range(4):
                            jj = jg * 4 + j
                            for i, (h0, hw) in enumerate(halves):
                                nc.tensor.matmul(
                                    ctps[i][:, 0:hw],
                                    attnT[:, jj : jj + 1],
                                    hb[:, j * N + h0 : j * N + h0 + hw],
                                    start=(jj == 0),
                                    stop=(jj == TJ - 1),
                                )
                    ctrow = smallp.tile([1, N], F32, tag="ctrow", bufs=2)
                    for i, (h0, hw) in enumerate(halves):
                        nc.vector.tensor_copy(ctrow[:, h0 : h0 + hw], ctps[i][:, 0:hw])
                    nc.sync.dma_start(out=ct_out[b : b + 1, :], in_=ctrow[:])

                # software pipeline: batch b's softmax/c_t is emitted after
                # batch b+1's first span-pair so PE has matmul work queued
                # while the softmax chain runs on ACT/DVE.
                pend = None
                deferred = []
                for b in range(B):
                    covrow = rowp.tile([1, T], F32, tag="covrow")
                    nc.sync.dma_start(out=covrow[:], in_=cov_d[b : b + 1, :])
                    covrow_b = rowp.tile([1, T], BF16, tag="covrowb")
                    nc.vector.tensor_copy(covrow_b[:], covrow[:])
                    scores_sb = rowp.tile([1, T], F32, tag="scores")
                    for pi, pair in enumerate(pairs):
                        hT = stage_pair(b, pair, hbfs[b])
                        pair_TM(b, pair, covrow_b, scores_sb, hT, deferred)
                        if pi == 0 and pend is not None:
                            phase_SC(*pend)
                            pend = None
                    while deferred:
                        deferred.pop(0)()
                    pend = (b, covrow, scores_sb, hbfs[b])
                phase_SC(*pend)

    return nc


_CACHE = {}


def _get_graph():
    if "nc" not in _CACHE:
        _enable_ldw_opt()
        nc = build_graph()
        nc.finalize()
        _CACHE["nc"] = nc
    return _CACHE["nc"]


def kernel(s_t_hat, h, enc_padding_mask, coverage, W_h, W_dec, b_dec, w_c, v, **_):
    from concourse.bass_utils import run_bass_kernel_spmd

    f = lambda x: np.ascontiguousarray(np.asarray(x), dtype=np.float32)
    s_t_hat, h = f(s_t_hat), f(h)
    enc_padding_mask, coverage = f(enc_padding_mask), f(coverage)
    W_h, W_dec = f(W_h), f(W_dec)
    b_dec, w_c, v = (
        f(b_dec).reshape(1, -1),
        f(w_c).reshape(1, -1),
        f(v).reshape(1, -1),
    )

    Btot = h.shape[0]
    Bper = Btot // N_CORES
    nc = _get_graph()
    in_maps = []
    for i in range(N_CORES):
        sl = slice(i * Bper, (i + 1) * Bper)
        in_maps.append(
            {
                "s_t_hat": s_t_hat[sl],
                "h": h[sl],
                "mask": enc_padding_mask[sl],
                "coverage": coverage[sl],
                "W_h": W_h,
                "W_dec": W_dec,
                "b_dec": b_dec,
                "w_c": w_c,
                "v": v,
            }
        )
    res = run_bass_kernel_spmd(
        nc,
        in_maps,
        core_ids=list(range(N_CORES)),
        trace=bool(os.environ.get("KERNEL_TRACE")),
    )
    kernel.last_results = res
    rs = res.results
    ct = np.concatenate([r["ct"] for r in rs], 0)
    attn = np.concatenate([r["attn"] for r in rs], 0)
    covnew = np.concatenate([r["covnew"] for r in rs], 0)
    return ct, attn, covnew
